# revision 25
# baseline (speedup 1.0000x reference)
"""Causal multi-head attention on 8 Trainium2 NeuronCores.

Problem: x[8,1024,768], 12 heads of d_head=64, causal softmax attention,
output projection. Sharding: data-parallel over batch (8 batch elements ==
8 cores), no collectives.

Per-core layout (zero on-device transposes):
  - host passes x.T [768,1024] bf16 (d_model on partitions)
  - qT/kT [768,1024] head-major rows  (d_head on partitions, seq on free)
  - scoresT[k, q] = kT_blk.T @ qT     (k on partitions, q on free); the two
    heads of a pair run as K=64 matmuls on opposite PE row-group halves
    (auto-derived tile_position) so they can stream concurrently on HW
  - v_aug [1024, 12*128]: per head 64 v columns + 64 ones columns; the AV
    matmul zT_psum = v_aug_blk.T @ exp(scoresT) then yields the softmax
    denominator (replicated) in psum partitions 64..127 for free
  - normalize with DVE reciprocal + tensor_mul (one PSUM operand)
  - out = zT.T @ W_O accumulated over head-pair chunks
Causal structure: only lower-triangular (k <= q) 128-blocks are computed;
the diagonal block is zeroed above the diagonal post-exp via affine_select.
exp runs without max subtraction (scores are O(1) by construction;
exp(-1e5) == 0 in fp32 matches the reference masking).

Schedule: the kernel is engine-balanced rather than phase-sequential, and
tuned for two measured HW effects the timeline sim does not model: (a) a
~50ns per-instruction dispatch/fetch tax (so instruction count is a
first-class cost: projection matmuls are grouped at the largest legal N,
the two heads share one 2-bank score PSUM tile so a single exp
instruction serves both, the causal mask is one 3D-AP affine_select, the
v_aug ones columns are one strided memset, and weights/x move in
whole-tensor DMAs), and (b) the ACT exp stream (~1.4ns/col) is the
per-iteration long pole of the attention loop, while the Q/K/V
projections are pure PE work with ACT idle.  So attention runs q-chunk
outer (two 512-wide q chunks; the z accumulators then need only 2 PSUM
banks, leaving 4 for the double-buffered 2-bank score tiles + 2 for
projection groups), scores run a 2-behind software pipeline ahead of the
AV matmuls, and projection groups interleave into the attention loop as
filler: q/k of pair p+1 inside attention of pair p, v for heads 0..7 /
8..11 inside the prologue / pair 0, the first half of the output
projection inside the last pair, and — across invocations — the next
rep's q/k pair-0 groups and v quad inside this rep's last pair and output
tail, with the next rep's input DMAs issued mid-rep on the idle SP queue
(the ACT/Pool queues are head-of-line blocked by exp/mask work until the
rep ends).  All matmul operands are bf16 (FWL hides the LDWEIGHTS phase;
fp32r stalls ~107ns per matmul on the weight load); PSUM accumulation is
fp32.  End-to-end rel err ~3e-3 vs the fp32 reference (tolerance 2e-2).
"""

from contextlib import ExitStack

import numpy as np

import concourse.mybir as mybir
import concourse.tile as tile
from concourse import bacc, bass_utils

F32 = mybir.dt.float32
BF = mybir.dt.bfloat16
MDT = BF

S = 1024        # seq len
D = 768         # d_model
H = 12          # heads
DH = 64         # d_head
P = 128         # partitions
KC = D // P     # 6 k-chunks of d_model
SB = S // P     # 8 seq blocks
PAIRS = H // 2  # 6 head pairs
VW = 2 * DH     # 128: v cols + ones cols per head in v_aug
N_CORES = 8


def make_pools(tc, ctx):
    return {
        "consts": ctx.enter_context(tc.tile_pool(name="consts", bufs=1)),
        "xw": ctx.enter_context(tc.tile_pool(name="xw", bufs=2)),
        "big": ctx.enter_context(tc.tile_pool(name="big", bufs=1)),
        "wqk": ctx.enter_context(tc.tile_pool(name="wqk", bufs=1)),
        "wo": ctx.enter_context(tc.tile_pool(name="wo", bufs=2)),
        "exp": ctx.enter_context(tc.tile_pool(name="exp", bufs=6)),
        "rec": ctx.enter_context(tc.tile_pool(name="rec", bufs=2)),
        "outb": ctx.enter_context(tc.tile_pool(name="outb", bufs=3)),
        "pp": ctx.enter_context(tc.tile_pool(name="ps", bufs=2, space="PSUM")),
        "ppz": ctx.enter_context(tc.tile_pool(name="psz", bufs=2, space="PSUM")),
        "ppj": ctx.enter_context(tc.tile_pool(name="psj", bufs=2, space="PSUM")),
    }


def issue_inputs(tc, pools, ins, first):
    """Allocate the double-buffered input tiles for one rep and issue their
    DMAs.  At kernel start (`first`) the transfers spread across the idle
    ACT/Pool/SP queues; mid-rep prefetch uses only the SP queue (the others
    are head-of-line blocked by the current rep's work)."""
    nc = tc.nc
    st = {}
    st["xt"] = pools["xw"].tile([P, KC * S], MDT, tag="xt", name="xt")
    st["wv"] = pools["xw"].tile([P, KC * D], MDT, tag="wv", name="wv")
    st["wqa"] = pools["wqk"].tile([P, PAIRS * KC * P], BF, tag="wqa", name="wqa")
    st["wka"] = pools["wqk"].tile([P, PAIRS * KC * P], BF, tag="wka", name="wka")
    st["bqt"] = pools["consts"].tile([P, PAIRS], F32, tag="bqt", name="bqt")
    st["bkt"] = pools["consts"].tile([P, PAIRS], F32, tag="bkt", name="bkt")
    st["bvb"] = pools["consts"].tile([P, D], F32, tag="bvb", name="bvb")
    big = pools["big"]
    st["qT"] = big.tile([P, PAIRS * S], MDT, tag="qT", name="qT")
    st["kT"] = big.tile([P, PAIRS * S], MDT, tag="kT", name="kT")
    st["zT"] = big.tile([P, PAIRS * S], MDT, tag="zT", name="zT")
    st["vaug"] = big.tile([P, SB * H * VW], MDT, tag="vaug", name="vaug")
    # ones columns of v_aug (cols 64..127 of each head block): one strided
    # memset over all 96 blocks
    nc.gpsimd.memset(
        st["vaug"][:].rearrange("p (b c) -> p b c", c=VW)[:, :, DH:VW], 1.0)

    # wq/wk whole-tensor (pair-contiguous host layout; one DMA each)
    nc.sync.dma_start(
        st["wqa"][:].rearrange("p (r c) -> p r c", c=KC * P),
        ins["wq"].rearrange("(r p) c -> p r c", p=P))
    nc.sync.dma_start(
        st["wka"][:].rearrange("p (r c) -> p r c", c=KC * P),
        ins["wk"].rearrange("(r p) c -> p r c", p=P))
    if first:
        # x in column-halves (all kc of half 0 first: the first projection
        # groups only need q < 512), spread over the idle ACT/Pool queues
        xq = [nc.scalar, nc.gpsimd]
        for half in range(2):
            for kc in range(KC):
                xq[kc % 2].dma_start(
                    st["xt"][:, kc * S + half * 512: kc * S + (half + 1) * 512],
                    ins["xT"][kc * P:(kc + 1) * P, half * 512:(half + 1) * 512])
        cq = nc.gpsimd
    else:
        nc.sync.dma_start(
            st["xt"][:].rearrange("p (k s) -> p k s", s=S),
            ins["xT"].rearrange("(k p) s -> p k s", p=P))
        cq = nc.sync
    cq.dma_start(st["bqt"][:], ins["bqt"].rearrange("(c p) x -> p (c x)", p=P))
    cq.dma_start(st["bkt"][:], ins["bkt"].rearrange("(c p) x -> p (c x)", p=P))
    cq.dma_start(st["bvb"][:], ins["bvb"][:])
    if first:
        for kc in range(KC):
            cq.dma_start(st["wv"][:, kc * D:(kc + 1) * D],
                         ins["wv"][kc * P:(kc + 1) * P, :])
    else:
        nc.sync.dma_start(
            st["wv"][:].rearrange("p (k d) -> p k d", d=D),
            ins["wv"].rearrange("(k p) d -> p k d", p=P))
    return st


def attention_kernel(tc, out_ap, ins, pools, st, prefetch):
    nc = tc.nc
    qT, kT, zT, vaug = st["qT"], st["kT"], st["zT"], st["vaug"]

    wo = pools["wo"].tile([P, KC * D], MDT, tag="wo")
    bob = pools["wo"].tile([P, D], F32, tag="bob")  # b_O broadcast to 128 rows

    def emit_wo_dma():
        nc.gpsimd.dma_start(bob[:], ins["bob"][:])
        nc.sync.dma_start(wo[:].rearrange("p (k d) -> p k d", d=D),
                          ins["wo"].rearrange("(k p) d -> p k d", p=P))

    # ---- projection groups (6 accumulating matmuls + one DVE drain); sx
    # selects the rep whose tiles are written (the next rep's first
    # projections run inside this rep's last attention pair / output tail)
    def emit_qk_group(sx, p, half, which):
        w_sb = sx["wqa"] if which == 0 else sx["wka"]
        b_sb = sx["bqt"] if which == 0 else sx["bkt"]
        dst = sx["qT"] if which == 0 else sx["kT"]
        ps = pools["ppj"].tile([P, 512], F32, tag="psj")
        for kc in range(KC):
            nc.tensor.matmul(
                ps[:],
                lhsT=w_sb[:, p * KC * P + kc * P: p * KC * P + (kc + 1) * P],
                rhs=sx["xt"][:, kc * S + half * 512: kc * S + (half + 1) * 512],
                start=(kc == 0), stop=(kc == KC - 1))
        nc.vector.tensor_scalar_add(
            dst[:, p * S + half * 512: p * S + (half + 1) * 512],
            ps[:], b_sb[:, p:p + 1])

    def emit_v_group(sx, s, h0, nh):
        # v for heads h0..h0+nh at seq block s: psum [128, nh*64]
        ps = pools["ppj"].tile([P, nh * DH], F32, tag="psj", name=f"vps_{s}_{h0}")
        for kc in range(KC):
            nc.tensor.matmul(
                ps[:],
                lhsT=sx["xt"][:, kc * S + s * P: kc * S + (s + 1) * P],
                rhs=sx["wv"][:, kc * D + h0 * DH: kc * D + (h0 + nh) * DH],
                start=(kc == 0), stop=(kc == KC - 1))
        base = s * H * VW + h0 * VW
        dst3 = sx["vaug"][:, base: base + nh * VW].rearrange(
            "p (n c) -> p n c", c=VW)[:, :, 0:DH]
        src3 = ps[:].rearrange("p (n c) -> p n c", c=DH)
        bv3 = sx["bvb"][:, h0 * DH: (h0 + nh) * DH].rearrange(
            "p (n c) -> p n c", c=DH)
        nc.vector.tensor_add(dst3, src3, bv3)

    def qk_groups(sx, p):
        return [lambda h=half, w=which: emit_qk_group(sx, p, h, w)
                for half in range(2) for which in range(2)]

    def v_groups(sx, h0, nh):
        return [lambda s=s: emit_v_group(sx, s, h0, nh) for s in range(SB)]

    out_tiles = {}

    def outproj_group(s, cb):
        n_cols = 512 if cb == 0 else 256
        outb = out_tiles.get(s)
        if outb is None:
            outb = out_tiles[s] = pools["outb"].tile([P, D], F32, tag="outb",
                                                     name=f"outb_{s}")
        ps = pools["ppj"].tile([P, n_cols], F32, tag="psj", name=f"ops_{s}_{cb}")
        for p in range(PAIRS):
            nc.tensor.matmul(
                ps[:],
                lhsT=zT[:, p * S + s * P: p * S + (s + 1) * P],
                rhs=wo[:, p * D + cb * 512: p * D + cb * 512 + n_cols],
                start=(p == 0), stop=(p == PAIRS - 1))
        nc.vector.tensor_add(outb[:, cb * 512: cb * 512 + n_cols],
                             ps[:],
                             bob[:, cb * 512: cb * 512 + n_cols])
        if cb == 1:
            nc.sync.dma_start(out_ap[s * P:(s + 1) * P, :], outb[:])

    # ---- attention for pair p, q-chunk outer (two 512-wide chunks),
    # k-blocks inner, with `filler` groups interleaved one per (c, j)
    def emit_attn(p, filler):
        expt = {}
        zps = {}

        def emit_scores(c, j):
            qlo = max(512 * c, P * j)
            qhi = 512 * (c + 1)
            w = qhi - qlo
            # both heads' scores in one 2-bank psum tile (head o at cols
            # o*512..), so one exp instruction covers both (the per-instr
            # PSUM access latency on ACT is ~240ns)
            sps = pools["pp"].tile([P, 1024], F32, tag="ps",
                                   name=f"sps_{p}_{c}_{j}")
            for o in range(2):
                nc.tensor.matmul(
                    sps[:, o * 512: o * 512 + w],
                    lhsT=kT[o * DH:(o + 1) * DH, p * S + j * P: p * S + (j + 1) * P],
                    rhs=qT[o * DH:(o + 1) * DH, p * S + qlo: p * S + qhi],
                    start=True, stop=True)
            et = expt[c, j] = pools["exp"].tile([P, 2 * w], MDT, tag="exp",
                                                name=f"exp_{p}_{c}_{j}")
            nc.scalar.activation(
                et[:].rearrange("p (o c) -> p o c", c=w),
                sps[:].rearrange("p (o c) -> p o c", c=512)[:, :, 0:w],
                mybir.ActivationFunctionType.Exp, scale=0.125)
            if j >= 4 * c:
                # diagonal block at the piece start (both heads in one op):
                # keep iff col >= part
                dg = et[:].rearrange("p (o c) -> p o c", c=w)[:, :, 0:P]
                nc.gpsimd.affine_select(
                    out=dg, in_=dg,
                    compare_op=mybir.AluOpType.is_ge,
                    fill=0.0, base=0,
                    pattern=[[0, 2], [1, P]], channel_multiplier=-1)

        def emit_av(c, j):
            jmax = 4 * c + 3
            qlo = max(512 * c, P * j)
            off = qlo - 512 * c
            w = 512 * (c + 1) - qlo
            for o in range(2):
                n = 2 * p + o
                nc.tensor.matmul(
                    zps[c, o][:, off:off + w],
                    lhsT=vaug[:, j * H * VW + n * VW: j * H * VW + (n + 1) * VW],
                    rhs=expt[c, j][:, o * w: (o + 1) * w],
                    start=(j == 0), stop=(j == jmax))
            del expt[c, j]
            if j == jmax:
                for o in range(2):
                    rec = pools["rec"].tile([DH, 512], F32, tag="rec")
                    nc.vector.reciprocal(rec[:], zps[c, o][DH:P, 0:512])
                    nc.vector.tensor_mul(
                        zT[o * DH:(o + 1) * DH, p * S + c * 512: p * S + (c + 1) * 512],
                        zps[c, o][0:DH, 0:512], rec[:])

        # 2-behind software pipeline: scores for (c,j+2) are emitted before
        # the AV of (c,j), so the ACT exp (the per-iteration long pole at
        # ~1.4ns/col vs the PE's 0.83) never stalls the AV matmuls
        seq = [(c, j) for c in range(2) for j in range(4 * c + 4)]
        for c in range(2):
            for o in range(2):
                zps[c, o] = pools["ppz"].tile([P, 512], F32, tag="psz",
                                              name=f"zps_{p}_{c}_{o}")
        emit_scores(*seq[0])
        emit_scores(*seq[1])
        for i, (c, j) in enumerate(seq):
            if i + 2 < len(seq):
                emit_scores(*seq[i + 2])
            if i < len(filler):
                filler[i]()
            emit_av(c, j)
        for g in filler[len(seq):]:
            g()

    # ---- prologue: qk for pair 0, v for heads 0..7 (pairs 0-3), ordered so
    # the half-0 x columns (which arrive first) are consumed first.  A rep
    # whose predecessor already emitted these (cross-rep interleave) skips.
    if not st.get("pre"):
        qk0 = qk_groups(st, 0)
        vq = v_groups(st, 0, 8)
        for g in [qk0[0], qk0[1]] + vq[0:4] + [qk0[2], qk0[3]] + vq[4:8]:
            g()

    st_next = None
    for p in range(PAIRS):
        if p + 2 == PAIRS:
            emit_wo_dma()
            st_next = prefetch()
        if p == 0:
            filler = qk_groups(st, 1) + v_groups(st, 8, 4)
        elif p + 1 < PAIRS:
            filler = qk_groups(st, p + 1)
        else:
            # c0 slots: the next rep's first q/k projections (its inputs
            # were prefetched at p==4).  c1 slots: the first half of the
            # output projection — s<4 rows of zT are final once this pair's
            # chunk-0 normalize has run.
            nqk = qk_groups(st_next, 0) if st_next else 4 * [lambda: None]
            filler = nqk + [
                lambda s=s, cb=cb: outproj_group(s, cb)
                for s in range(4) for cb in range(2)]
        emit_attn(p, filler)

    # tail: remaining output projection, interleaved with the next rep's
    # v projection for heads 0..7 (pure PE filler for the zT/DVE stalls)
    nvq = v_groups(st_next, 0, 8) if st_next else 8 * [lambda: None]
    ti = 0
    for s in range(4, SB):
        for cb in range(2):
            outproj_group(s, cb)
            if ti < 8:
                nvq[ti]()
            ti += 1
    if st_next is not None:
        st_next["pre"] = True
    return st_next


_CACHED = {}


def build_program(reps=1):
    if reps in _CACHED:
        return _CACHED[reps]
    nc = bacc.Bacc("TRN2", target_bir_lowering=False, debug=False)
    ins = {
        "xT": nc.dram_tensor("xT", [D, S], MDT, kind="ExternalInput").ap(),
        "wq": nc.dram_tensor("wq", [D, D], BF, kind="ExternalInput").ap(),
        "wk": nc.dram_tensor("wk", [D, D], BF, kind="ExternalInput").ap(),
        "wv": nc.dram_tensor("wv", [D, D], MDT, kind="ExternalInput").ap(),
        "wo": nc.dram_tensor("wo", [D, D], MDT, kind="ExternalInput").ap(),
        "bqt": nc.dram_tensor("bqt", [D, 1], F32, kind="ExternalInput").ap(),
        "bkt": nc.dram_tensor("bkt", [D, 1], F32, kind="ExternalInput").ap(),
        "bvb": nc.dram_tensor("bvb", [P, D], F32, kind="ExternalInput").ap(),
        "bob": nc.dram_tensor("bob", [P, D], F32, kind="ExternalInput").ap(),
    }
    out = nc.dram_tensor("out", [S, D], F32, kind="ExternalOutput").ap()
    with tile.TileContext(nc) as tc, ExitStack() as ctx:
        pools = make_pools(tc, ctx)
        st = issue_inputs(tc, pools, ins, first=True)
        for r in range(reps):

            def prefetch(r=r):
                if r + 1 < reps:
                    return issue_inputs(tc, pools, ins, first=False)
                return None

            st = attention_kernel(tc, out, ins, pools, st, prefetch)
    nc.compile()
    _CACHED[reps] = nc
    return nc


def make_in_maps(normalized_resid_pre, W_Q, W_K, W_V, W_O, b_Q, b_K, b_V, b_O):
    x = np.asarray(normalized_resid_pre, np.float32)
    import ml_dtypes
    bf = ml_dtypes.bfloat16

    def pairwise(w):
        # [d_model, head-major] -> pair-contiguous SBUF layout
        # out[pair*128 + p, kc*128 + n] = w[kc*128 + p, pair*128 + n]
        t = np.asarray(w, np.float32).reshape(KC, P, PAIRS, P)
        return np.ascontiguousarray(
            t.transpose(2, 1, 0, 3).reshape(D, D).astype(bf))

    wq_m = pairwise(np.asarray(W_Q, np.float32).transpose(1, 0, 2).reshape(D, D))
    wk_m = pairwise(np.asarray(W_K, np.float32).transpose(1, 0, 2).reshape(D, D))
    wv_m = np.ascontiguousarray(
        np.asarray(W_V, np.float32).transpose(1, 0, 2).reshape(D, D).astype(bf))
    wo_m = np.ascontiguousarray(
        np.asarray(W_O, np.float32).reshape(D, D).astype(bf))
    bq_m = np.asarray(b_Q, np.float32).reshape(D, 1)
    bk_m = np.asarray(b_K, np.float32).reshape(D, 1)
    bv_m = np.ascontiguousarray(np.broadcast_to(
        np.asarray(b_V, np.float32).reshape(1, D), (P, D)))
    bo_m = np.ascontiguousarray(np.broadcast_to(
        np.asarray(b_O, np.float32).reshape(1, D), (P, D)))
    in_maps = []
    for b in range(N_CORES):
        in_maps.append({
            "xT": np.ascontiguousarray(x[b].T).astype(bf),
            "wq": wq_m, "wk": wk_m, "wv": wv_m, "wo": wo_m,
            "bqt": bq_m, "bkt": bk_m, "bvb": bv_m, "bob": bo_m,
        })
    return in_maps


def kernel(**inputs):
    nc = build_program()
    in_maps = make_in_maps(**inputs)
    res = bass_utils.run_bass_kernel_spmd(nc, in_maps, list(range(N_CORES)))
    return np.stack([r["out"] for r in res.results])


# revision 26
# speedup vs baseline: 1.0722x; 1.0722x over previous
"""Causal multi-head attention on 8 Trainium2 NeuronCores.

Problem: x[8,1024,768], 12 heads of d_head=64, causal softmax attention,
output projection. Sharding: data-parallel over batch (8 batch elements ==
8 cores), no collectives.

Per-core layout (zero on-device transposes):
  - host passes x.T [768,1024] bf16 (d_model on partitions)
  - qT/kT [768,1024] head-major rows  (d_head on partitions, seq on free)
  - scoresT[k, q] = kT_blk.T @ qT     (k on partitions, q on free); the two
    heads of a pair run as K=64 matmuls on opposite PE row-group halves
    (auto-derived tile_position) so they can stream concurrently on HW
  - v_aug [1024, 12*128]: per head 64 v columns + 64 ones columns; the AV
    matmul zT_psum = v_aug_blk.T @ exp(scoresT) then yields the softmax
    denominator (replicated) in psum partitions 64..127 for free
  - normalize with DVE reciprocal + tensor_mul (one PSUM operand)
  - out = zT.T @ W_O accumulated over head-pair chunks
Causal structure: only lower-triangular (k <= q) 128-blocks are computed;
the diagonal block is zeroed above the diagonal post-exp via affine_select.
exp runs without max subtraction (scores are O(1) by construction;
exp(-1e5) == 0 in fp32 matches the reference masking).

Schedule: the kernel is engine-balanced rather than phase-sequential, and
tuned for two measured HW effects the timeline sim does not model: (a) a
~50ns per-instruction dispatch/fetch tax (so instruction count is a
first-class cost: projection matmuls are grouped at the largest legal N,
the two heads share one 2-bank score PSUM tile so a single exp
instruction serves both, the causal mask is one 3D-AP affine_select, the
v_aug ones columns are one strided memset, and weights/x move in
whole-tensor DMAs), and (b) the ACT exp stream (~1.4ns/col) is the
per-iteration long pole of the attention loop, while the Q/K/V
projections are pure PE work with ACT idle.  So attention runs q-chunk
outer (two 512-wide q chunks; the z accumulators then need only 2 PSUM
banks, leaving 4 for the double-buffered 2-bank score tiles + 2 for
projection groups), scores run a 2-behind software pipeline ahead of the
AV matmuls, and projection groups interleave into the attention loop as
filler: q/k of pair p+1 inside attention of pair p, v for heads 0..7 /
8..11 inside the prologue / pair 0, the first half of the output
projection inside the last pair, and — across invocations — the next
rep's q/k pair-0 groups and v quad inside this rep's last pair and output
tail, with the next rep's input DMAs issued mid-rep on the idle SP queue
(the ACT/Pool queues are head-of-line blocked by exp/mask work until the
rep ends).  All matmul operands are bf16 (FWL hides the LDWEIGHTS phase;
fp32r stalls ~107ns per matmul on the weight load); PSUM accumulation is
fp32.  End-to-end rel err ~3e-3 vs the fp32 reference (tolerance 2e-2).
"""

from contextlib import ExitStack

import numpy as np

import concourse.mybir as mybir
import concourse.tile as tile
from concourse import bacc, bass_utils

F32 = mybir.dt.float32
BF = mybir.dt.bfloat16
MDT = BF

S = 1024        # seq len
D = 768         # d_model
H = 12          # heads
DH = 64         # d_head
P = 128         # partitions
KC = D // P     # 6 k-chunks of d_model
SB = S // P     # 8 seq blocks
PAIRS = H // 2  # 6 head pairs
VW = 2 * DH     # 128: v cols + ones cols per head in v_aug
N_CORES = 8


def make_pools(tc, ctx):
    return {
        "consts": ctx.enter_context(tc.tile_pool(name="consts", bufs=1)),
        "xw": ctx.enter_context(tc.tile_pool(name="xw", bufs=2)),
        "big": ctx.enter_context(tc.tile_pool(name="big", bufs=1)),
        "wqk": ctx.enter_context(tc.tile_pool(name="wqk", bufs=1)),
        "wo": ctx.enter_context(tc.tile_pool(name="wo", bufs=2)),
        "exp": ctx.enter_context(tc.tile_pool(name="exp", bufs=8)),
        "rec": ctx.enter_context(tc.tile_pool(name="rec", bufs=4)),
        "outb": ctx.enter_context(tc.tile_pool(name="outb", bufs=3)),
        "pp": ctx.enter_context(tc.tile_pool(name="ps", bufs=2, space="PSUM")),
        "ppz": ctx.enter_context(tc.tile_pool(name="psz", bufs=2, space="PSUM")),
        "ppj": ctx.enter_context(tc.tile_pool(name="psj", bufs=2, space="PSUM")),
    }


def issue_inputs(tc, pools, ins, first):
    """Allocate the double-buffered input tiles for one rep and issue their
    DMAs.  At kernel start (`first`) the transfers spread across the idle
    ACT/Pool/SP queues; mid-rep prefetch uses only the SP queue (the others
    are head-of-line blocked by the current rep's work)."""
    nc = tc.nc
    st = {}
    st["xt"] = pools["xw"].tile([P, KC * S], MDT, tag="xt", name="xt")
    st["wv"] = pools["xw"].tile([P, KC * D], MDT, tag="wv", name="wv")
    st["wqa"] = pools["wqk"].tile([P, PAIRS * KC * P], BF, tag="wqa", name="wqa")
    st["wka"] = pools["wqk"].tile([P, PAIRS * KC * P], BF, tag="wka", name="wka")
    st["bqt"] = pools["consts"].tile([P, PAIRS], F32, tag="bqt", name="bqt")
    st["bkt"] = pools["consts"].tile([P, PAIRS], F32, tag="bkt", name="bkt")
    st["bvb"] = pools["consts"].tile([P, D], F32, tag="bvb", name="bvb")
    big = pools["big"]
    st["qT"] = big.tile([P, PAIRS * S], MDT, tag="qT", name="qT")
    st["kT"] = big.tile([P, PAIRS * S], MDT, tag="kT", name="kT")
    st["zT"] = big.tile([P, PAIRS * S], MDT, tag="zT", name="zT")
    st["vaug"] = big.tile([P, SB * H * VW], MDT, tag="vaug", name="vaug")
    # ones columns of v_aug (cols 64..127 of each head block): one strided
    # memset over all 96 blocks
    nc.gpsimd.memset(
        st["vaug"][:].rearrange("p (b c) -> p b c", c=VW)[:, :, DH:VW], 1.0)

    # wq/wk whole-tensor (pair-contiguous host layout; one DMA each)
    nc.sync.dma_start(
        st["wqa"][:].rearrange("p (r c) -> p r c", c=KC * P),
        ins["wq"].rearrange("(r p) c -> p r c", p=P))
    nc.sync.dma_start(
        st["wka"][:].rearrange("p (r c) -> p r c", c=KC * P),
        ins["wk"].rearrange("(r p) c -> p r c", p=P))
    if first:
        # x in column-halves (all kc of half 0 first: the first projection
        # groups only need q < 512), spread over the idle ACT/Pool queues
        xq = [nc.scalar, nc.gpsimd]
        for half in range(2):
            for kc in range(KC):
                xq[kc % 2].dma_start(
                    st["xt"][:, kc * S + half * 512: kc * S + (half + 1) * 512],
                    ins["xT"][kc * P:(kc + 1) * P, half * 512:(half + 1) * 512])
        cq = nc.gpsimd
    else:
        nc.sync.dma_start(
            st["xt"][:].rearrange("p (k s) -> p k s", s=S),
            ins["xT"].rearrange("(k p) s -> p k s", p=P))
        cq = nc.sync
    cq.dma_start(st["bqt"][:], ins["bqt"].rearrange("(c p) x -> p (c x)", p=P))
    cq.dma_start(st["bkt"][:], ins["bkt"].rearrange("(c p) x -> p (c x)", p=P))
    cq.dma_start(st["bvb"][:], ins["bvb"][:])
    if first:
        for kc in range(KC):
            cq.dma_start(st["wv"][:, kc * D:(kc + 1) * D],
                         ins["wv"][kc * P:(kc + 1) * P, :])
    else:
        nc.sync.dma_start(
            st["wv"][:].rearrange("p (k d) -> p k d", d=D),
            ins["wv"].rearrange("(k p) d -> p k d", p=P))
    return st


def attention_kernel(tc, out_ap, ins, pools, st, prefetch):
    nc = tc.nc
    qT, kT, zT, vaug = st["qT"], st["kT"], st["zT"], st["vaug"]

    wo = pools["wo"].tile([P, KC * D], MDT, tag="wo")
    bob = pools["wo"].tile([P, D], F32, tag="bob")  # b_O broadcast to 128 rows

    def emit_wo_dma():
        nc.gpsimd.dma_start(bob[:], ins["bob"][:])
        nc.sync.dma_start(wo[:].rearrange("p (k d) -> p k d", d=D),
                          ins["wo"].rearrange("(k p) d -> p k d", p=P))

    # ---- projection groups (6 accumulating matmuls + one DVE drain); sx
    # selects the rep whose tiles are written (the next rep's first
    # projections run inside this rep's last attention pair / output tail)
    def emit_qk_group(sx, p, half, which):
        w_sb = sx["wqa"] if which == 0 else sx["wka"]
        b_sb = sx["bqt"] if which == 0 else sx["bkt"]
        dst = sx["qT"] if which == 0 else sx["kT"]
        ps = pools["ppj"].tile([P, 512], F32, tag="psj")
        for kc in range(KC):
            nc.tensor.matmul(
                ps[:],
                lhsT=w_sb[:, p * KC * P + kc * P: p * KC * P + (kc + 1) * P],
                rhs=sx["xt"][:, kc * S + half * 512: kc * S + (half + 1) * 512],
                start=(kc == 0), stop=(kc == KC - 1))
        nc.vector.tensor_scalar_add(
            dst[:, p * S + half * 512: p * S + (half + 1) * 512],
            ps[:], b_sb[:, p:p + 1])

    def emit_v_group(sx, s, h0, nh):
        # v for heads h0..h0+nh at seq block s: psum [128, nh*64]
        ps = pools["ppj"].tile([P, nh * DH], F32, tag="psj", name=f"vps_{s}_{h0}")
        for kc in range(KC):
            nc.tensor.matmul(
                ps[:],
                lhsT=sx["xt"][:, kc * S + s * P: kc * S + (s + 1) * P],
                rhs=sx["wv"][:, kc * D + h0 * DH: kc * D + (h0 + nh) * DH],
                start=(kc == 0), stop=(kc == KC - 1))
        base = s * H * VW + h0 * VW
        dst3 = sx["vaug"][:, base: base + nh * VW].rearrange(
            "p (n c) -> p n c", c=VW)[:, :, 0:DH]
        src3 = ps[:].rearrange("p (n c) -> p n c", c=DH)
        bv3 = sx["bvb"][:, h0 * DH: (h0 + nh) * DH].rearrange(
            "p (n c) -> p n c", c=DH)
        nc.vector.tensor_add(dst3, src3, bv3)

    def qk_groups(sx, p):
        return [lambda h=half, w=which: emit_qk_group(sx, p, h, w)
                for half in range(2) for which in range(2)]

    def v_groups(sx, h0, nh):
        return [lambda s=s: emit_v_group(sx, s, h0, nh) for s in range(SB)]

    out_tiles = {}

    def outproj_group(s, cb):
        n_cols = 512 if cb == 0 else 256
        outb = out_tiles.get(s)
        if outb is None:
            outb = out_tiles[s] = pools["outb"].tile([P, D], F32, tag="outb",
                                                     name=f"outb_{s}")
        ps = pools["ppj"].tile([P, n_cols], F32, tag="psj", name=f"ops_{s}_{cb}")
        for p in range(PAIRS):
            nc.tensor.matmul(
                ps[:],
                lhsT=zT[:, p * S + s * P: p * S + (s + 1) * P],
                rhs=wo[:, p * D + cb * 512: p * D + cb * 512 + n_cols],
                start=(p == 0), stop=(p == PAIRS - 1))
        nc.vector.tensor_add(outb[:, cb * 512: cb * 512 + n_cols],
                             ps[:],
                             bob[:, cb * 512: cb * 512 + n_cols])
        if cb == 1:
            nc.sync.dma_start(out_ap[s * P:(s + 1) * P, :], outb[:])

    # ---- attention for pair p, q-chunk outer (two 512-wide chunks),
    # k-blocks inner, with `filler` groups interleaved one per (c, j)
    def emit_attn(p, filler):
        expt = {}
        zps = {}

        def emit_scores(c, j):
            qlo = max(512 * c, P * j)
            qhi = 512 * (c + 1)
            w = qhi - qlo
            # both heads' scores in one 2-bank psum tile (head o at cols
            # o*512..), so one exp instruction covers both (the per-instr
            # PSUM access latency on ACT is ~240ns)
            sps = pools["pp"].tile([P, 1024], F32, tag="ps",
                                   name=f"sps_{p}_{c}_{j}")
            for o in range(2):
                nc.tensor.matmul(
                    sps[:, o * 512: o * 512 + w],
                    lhsT=kT[o * DH:(o + 1) * DH, p * S + j * P: p * S + (j + 1) * P],
                    rhs=qT[o * DH:(o + 1) * DH, p * S + qlo: p * S + qhi],
                    start=True, stop=True)
            et = expt[c, j] = pools["exp"].tile([P, 2 * w], MDT, tag="exp",
                                                name=f"exp_{p}_{c}_{j}")
            nc.scalar.activation(
                et[:].rearrange("p (o c) -> p o c", c=w),
                sps[:].rearrange("p (o c) -> p o c", c=512)[:, :, 0:w],
                mybir.ActivationFunctionType.Exp, scale=0.125)
            if j >= 4 * c:
                # diagonal block at the piece start (both heads in one op):
                # keep iff col >= part
                dg = et[:].rearrange("p (o c) -> p o c", c=w)[:, :, 0:P]
                nc.gpsimd.affine_select(
                    out=dg, in_=dg,
                    compare_op=mybir.AluOpType.is_ge,
                    fill=0.0, base=0,
                    pattern=[[0, 2], [1, P]], channel_multiplier=-1)

        def emit_av(c, j):
            jmax = 4 * c + 3
            qlo = max(512 * c, P * j)
            off = qlo - 512 * c
            w = 512 * (c + 1) - qlo
            for o in range(2):
                n = 2 * p + o
                nc.tensor.matmul(
                    zps[c, o][:, off:off + w],
                    lhsT=vaug[:, j * H * VW + n * VW: j * H * VW + (n + 1) * VW],
                    rhs=expt[c, j][:, o * w: (o + 1) * w],
                    start=(j == 0), stop=(j == jmax))
            del expt[c, j]
            if j == jmax:
                for o in range(2):
                    rec = pools["rec"].tile([DH, 512], F32, tag="rec")
                    nc.vector.reciprocal(rec[:], zps[c, o][DH:P, 0:512])
                    nc.vector.tensor_mul(
                        zT[o * DH:(o + 1) * DH, p * S + c * 512: p * S + (c + 1) * 512],
                        zps[c, o][0:DH, 0:512], rec[:])

        # 2-behind software pipeline: scores for (c,j+2) are emitted before
        # the AV of (c,j), so the ACT exp (the per-iteration long pole at
        # ~1.4ns/col vs the PE's 0.83) never stalls the AV matmuls
        seq = [(c, j) for c in range(2) for j in range(4 * c + 4)]
        for c in range(2):
            for o in range(2):
                zps[c, o] = pools["ppz"].tile([P, 512], F32, tag="psz",
                                              name=f"zps_{p}_{c}_{o}")
        emit_scores(*seq[0])
        emit_scores(*seq[1])
        for i, (c, j) in enumerate(seq):
            if i + 2 < len(seq):
                emit_scores(*seq[i + 2])
            if i < len(filler):
                filler[i]()
            emit_av(c, j)
        for g in filler[len(seq):]:
            g()

    # ---- prologue: qk for pair 0, v for heads 0..7 (pairs 0-3), ordered so
    # the half-0 x columns (which arrive first) are consumed first.  A rep
    # whose predecessor already emitted these (cross-rep interleave) skips.
    if not st.get("pre"):
        qk0 = qk_groups(st, 0)
        vq = v_groups(st, 0, 8)
        for g in [qk0[0], qk0[1]] + vq[0:4] + [qk0[2], qk0[3]] + vq[4:8]:
            g()

    st_next = None
    for p in range(PAIRS):
        if p + 2 == PAIRS:
            emit_wo_dma()
            st_next = prefetch()
        if p == 0:
            filler = qk_groups(st, 1) + v_groups(st, 8, 4)
        elif p + 1 < PAIRS:
            filler = qk_groups(st, p + 1)
        else:
            # c0 slots: the next rep's first q/k projections (its inputs
            # were prefetched at p==4).  c1 slots: the first half of the
            # output projection — s<4 rows of zT are final once this pair's
            # chunk-0 normalize has run.
            nqk = qk_groups(st_next, 0) if st_next else 4 * [lambda: None]
            filler = nqk + [
                lambda s=s, cb=cb: outproj_group(s, cb)
                for s in range(4) for cb in range(2)]
        emit_attn(p, filler)

    # tail: remaining output projection, interleaved with the next rep's
    # v projection for heads 0..7 (pure PE filler for the zT/DVE stalls)
    nvq = v_groups(st_next, 0, 8) if st_next else 8 * [lambda: None]
    ti = 0
    for s in range(4, SB):
        for cb in range(2):
            outproj_group(s, cb)
            if ti < 8:
                nvq[ti]()
            ti += 1
    if st_next is not None:
        st_next["pre"] = True
    return st_next


_CACHED = {}


def build_program(reps=1):
    if reps in _CACHED:
        return _CACHED[reps]
    nc = bacc.Bacc("TRN2", target_bir_lowering=False, debug=False)
    ins = {
        "xT": nc.dram_tensor("xT", [D, S], MDT, kind="ExternalInput").ap(),
        "wq": nc.dram_tensor("wq", [D, D], BF, kind="ExternalInput").ap(),
        "wk": nc.dram_tensor("wk", [D, D], BF, kind="ExternalInput").ap(),
        "wv": nc.dram_tensor("wv", [D, D], MDT, kind="ExternalInput").ap(),
        "wo": nc.dram_tensor("wo", [D, D], MDT, kind="ExternalInput").ap(),
        "bqt": nc.dram_tensor("bqt", [D, 1], F32, kind="ExternalInput").ap(),
        "bkt": nc.dram_tensor("bkt", [D, 1], F32, kind="ExternalInput").ap(),
        "bvb": nc.dram_tensor("bvb", [P, D], F32, kind="ExternalInput").ap(),
        "bob": nc.dram_tensor("bob", [P, D], F32, kind="ExternalInput").ap(),
    }
    out = nc.dram_tensor("out", [S, D], F32, kind="ExternalOutput").ap()
    with tile.TileContext(nc) as tc, ExitStack() as ctx:
        pools = make_pools(tc, ctx)
        st = issue_inputs(tc, pools, ins, first=True)
        for r in range(reps):

            def prefetch(r=r):
                if r + 1 < reps:
                    return issue_inputs(tc, pools, ins, first=False)
                return None

            st = attention_kernel(tc, out, ins, pools, st, prefetch)
    nc.compile()
    _CACHED[reps] = nc
    return nc


def make_in_maps(normalized_resid_pre, W_Q, W_K, W_V, W_O, b_Q, b_K, b_V, b_O):
    x = np.asarray(normalized_resid_pre, np.float32)
    import ml_dtypes
    bf = ml_dtypes.bfloat16

    def pairwise(w):
        # [d_model, head-major] -> pair-contiguous SBUF layout
        # out[pair*128 + p, kc*128 + n] = w[kc*128 + p, pair*128 + n]
        t = np.asarray(w, np.float32).reshape(KC, P, PAIRS, P)
        return np.ascontiguousarray(
            t.transpose(2, 1, 0, 3).reshape(D, D).astype(bf))

    wq_m = pairwise(np.asarray(W_Q, np.float32).transpose(1, 0, 2).reshape(D, D))
    wk_m = pairwise(np.asarray(W_K, np.float32).transpose(1, 0, 2).reshape(D, D))
    wv_m = np.ascontiguousarray(
        np.asarray(W_V, np.float32).transpose(1, 0, 2).reshape(D, D).astype(bf))
    wo_m = np.ascontiguousarray(
        np.asarray(W_O, np.float32).reshape(D, D).astype(bf))
    bq_m = np.asarray(b_Q, np.float32).reshape(D, 1)
    bk_m = np.asarray(b_K, np.float32).reshape(D, 1)
    bv_m = np.ascontiguousarray(np.broadcast_to(
        np.asarray(b_V, np.float32).reshape(1, D), (P, D)))
    bo_m = np.ascontiguousarray(np.broadcast_to(
        np.asarray(b_O, np.float32).reshape(1, D), (P, D)))
    in_maps = []
    for b in range(N_CORES):
        in_maps.append({
            "xT": np.ascontiguousarray(x[b].T).astype(bf),
            "wq": wq_m, "wk": wk_m, "wv": wv_m, "wo": wo_m,
            "bqt": bq_m, "bkt": bk_m, "bvb": bv_m, "bob": bo_m,
        })
    return in_maps


def kernel(**inputs):
    nc = build_program()
    in_maps = make_in_maps(**inputs)
    res = bass_utils.run_bass_kernel_spmd(nc, in_maps, list(range(N_CORES)))
    return np.stack([r["out"] for r in res.results])
